# revision 1
# baseline (speedup 1.0000x reference)
"""Trainium2 Bass kernel for the Chebyshev atomic descriptor (gnn_message_passing).

Contract: kernel(**inputs) takes FULL unsharded inputs (positions [20000,3] f32,
species_idx [20000] i32, neighbor_idx [480000] i32) and returns the full
[20000, 52] f32 feature array. Internally shards atoms across 8 NeuronCores
(data-parallel over N), gathers neighbor data on-device via indirect DMA, and
concatenates per-core outputs on host.
"""

import math
from contextlib import ExitStack

import numpy as np

import bass_rust
import concourse.bass as bass
import concourse.bacc as bacc
import concourse.tile as tile
from concourse import mybir
from concourse.bass_utils import run_bass_kernel_spmd

F32 = mybir.dt.float32
I32 = mybir.dt.int32
I16 = mybir.dt.int16
Alu = mybir.AluOpType
Act = mybir.ActivationFunctionType
AX = mybir.AxisListType

# ---- problem constants (hardcoded per harness contract) ----
N = 20000
K = 24
NCORES = 8
NPAD = 20480            # padded N, divisible by NCORES*128*G
NPC = NPAD // NCORES    # atoms per core = 2560
PT = 128                # partitions
G = 10                  # atoms per partition per supertile
SUP = NPC // (PT * G)   # supertiles per core = 2
KG = K * G              # neighbor slots per partition per supertile = 240
RAD_ORDER = 16
ANG_ORDER = 8
RAD_CUT = 8.0
ANG_CUT = 6.5
MIN_CUT = 0.55
DG = 12                 # circular-distance groups d=1..12
PAIR = DG * K           # 288 pair columns per atom (d=12 double-counted, half-weighted)
PAIRG = PAIR * G        # 2880
FEAT = 52
ROWE = 64               # padded table row: 64 f32 = 256B (dma_gather granularity)
GQ = 8                  # dma_gather calls per supertile
NRAD = RAD_ORDER + 1    # 17
NANG = ANG_ORDER + 1    # 9

HALF_PI = math.pi / 2.0
# x = 2*(d - MIN_CUT)/(RAD_CUT - MIN_CUT) - 1 = d*AX_ + BX_
AX_ = 2.0 / (RAD_CUT - MIN_CUT)
BX_ = -2.0 * MIN_CUT / (RAD_CUT - MIN_CUT) - 1.0


def view(ap, off, dims):
    """Custom free-dim view of a tile AP: keep the partition entry, replace the
    free dims with explicit [step, count] pairs (supports step-0 broadcasts and
    overlapping windows), shift the in-partition element offset by `off`."""
    base = list(ap.ap[0])
    return bass_rust.AP(ap.tensor, ap.offset + off, [base] + [list(d) for d in dims])


def build_supertile(nc, io, kp, app, s, pos4, idx16, pself, feat, half_pi, dbg=None):
    base = s * PT * G  # first atom (core-local) of this supertile

    # ---- loads ----
    ps = io.tile([PT, 4 * G], F32, tag="ps")
    nc.sync.dma_start(
        out=ps[:],
        in_=pself[base : base + PT * G, :].rearrange("(p g) c -> p (g c)", p=PT),
    )
    # gather neighbor [x,y,z,s] rows via dma_gather (256B table rows), then
    # compact the leading 16B of each gathered row into pn [PT, KG*4]
    pn = io.tile([PT, KG * 4], F32, tag="pn")
    CQ = KG // GQ  # gathered (g,k) chunks per dma_gather call
    NIDX = CQ * PT
    for q in range(GQ):
        idx_t = io.tile([PT, NIDX // 16], I16, tag="idx_t")
        row0 = (s * GQ + q) * PT
        nc.sync.dma_start(out=idx_t[:], in_=idx16[row0 : row0 + PT, :])
        pnw = io.tile([PT, CQ * ROWE], F32, tag="pnw")
        nc.gpsimd.dma_gather(
            out_ap=view(pnw[:], 0, [[ROWE, CQ], [1, ROWE]]),
            in_ap=pos4,
            idxs_ap=idx_t[:],
            num_idxs=NIDX,
            num_idxs_reg=NIDX,
            elem_size=ROWE,
            single_packet=False,
        )
        nc.scalar.copy(
            out=view(pn[:], q * CQ * 4, [[1, CQ * 4]]),
            in_=view(pnw[:], 0, [[ROWE, CQ], [1, 4]]),
        )

    # ---- K-space prep (free dims (G, K) = 240 elems) ----
    # rvec = pos_nbr - pos_self
    r_c = []
    for c in range(3):
        r = kp.tile([PT, KG], F32, tag=f"r{c}")
        nc.vector.tensor_tensor(
            out=r[:].rearrange("p (g k) -> p g k", g=G),
            in0=view(pn[:], c, [[4 * K, G], [4, K]]),
            in1=view(ps[:], c, [[4, G], [0, K]]),
            op=Alu.subtract,
        )
        r_c.append(r)
    # d2 = rx^2 + ry^2 + rz^2  (squares on ACT, adds on DVE)
    sq = []
    for c in range(3):
        q = kp.tile([PT, KG], F32, tag=f"sq{c}")
        nc.scalar.activation(q[:], r_c[c][:], Act.Square)
        sq.append(q)
    d2 = kp.tile([PT, KG], F32, tag="d2")
    nc.vector.tensor_tensor(out=d2[:], in0=sq[0][:], in1=sq[1][:], op=Alu.add)
    nc.vector.tensor_tensor(out=d2[:], in0=d2[:], in1=sq[2][:], op=Alu.add)
    # clamp to avoid rsqrt(0); masked-out anyway (d <= MIN_CUT)
    nc.vector.tensor_scalar_max(d2[:], d2[:], 1e-18)
    dd = kp.tile([PT, KG], F32, tag="dd")
    nc.scalar.sqrt(dd[:], d2[:])
    rinv = kp.tile([PT, KG], F32, tag="rinv")
    nc.vector.reciprocal(rinv[:], dd[:])

    # unit vectors into extended (wrap-around) buffers [G, 36]
    ue = []
    for c in range(3):
        e = kp.tile([PT, 36 * G], F32, tag=f"ue{c}")
        nc.vector.tensor_tensor(
            out=view(e[:], 0, [[36, G], [1, K]]),
            in0=r_c[c][:].rearrange("p (g k) -> p g k", g=G),
            in1=rinv[:].rearrange("p (g k) -> p g k", g=G),
            op=Alu.mult,
        )
        ue.append(e)

    # masks: m2 = (d > MIN_CUT); m1h = 0.5*(d <= RAD_CUT); a1h = 0.5*(d <= ANG_CUT)
    m2 = kp.tile([PT, KG], F32, tag="m2")
    nc.vector.tensor_scalar(
        out=m2[:], in0=dd[:], scalar1=MIN_CUT, scalar2=None, op0=Alu.is_gt
    )
    m1h = kp.tile([PT, KG], F32, tag="m1h")
    nc.vector.tensor_scalar(
        out=m1h[:], in0=dd[:], scalar1=RAD_CUT, scalar2=0.5, op0=Alu.is_le, op1=Alu.mult
    )
    a1h = kp.tile([PT, KG], F32, tag="a1h")
    nc.vector.tensor_scalar(
        out=a1h[:], in0=dd[:], scalar1=ANG_CUT, scalar2=0.5, op0=Alu.is_le, op1=Alu.mult
    )
    # cos cutoffs via sin(pi/2 - pi*min(d,rc)/rc) = cos(pi*d/rc) for in-mask d.
    # Clamping d at rc keeps the sin argument in [-pi/2, pi/2] (ACT table
    # domain); clamped out-of-mask values give fc=0 and are masked anyway.
    dcr = kp.tile([PT, KG], F32, tag="dcr")
    nc.vector.tensor_scalar_min(dcr[:], dd[:], RAD_CUT)
    grad = kp.tile([PT, KG], F32, tag="grad")
    nc.scalar.activation(
        grad[:], dcr[:], Act.Sin, bias=half_pi[:], scale=-math.pi / RAD_CUT
    )
    dca = kp.tile([PT, KG], F32, tag="dca")
    nc.vector.tensor_scalar_min(dca[:], dd[:], ANG_CUT)
    gang = kp.tile([PT, KG], F32, tag="gang")
    nc.scalar.activation(
        gang[:], dca[:], Act.Sin, bias=half_pi[:], scale=-math.pi / ANG_CUT
    )

    # radial weights wr = fc*m = mh*(grad+1) (DVE)
    Sr0a = kp.tile([PT, KG], F32, tag="Sr0a")
    wr = Sr0a[:]
    mh = kp.tile([PT, KG], F32, tag="mh")
    nc.vector.tensor_tensor(out=mh[:], in0=m1h[:], in1=m2[:], op=Alu.mult)
    nc.vector.tensor_tensor(out=wr, in0=mh[:], in1=grad[:], op=Alu.mult)
    nc.vector.tensor_tensor(out=wr, in0=wr, in1=mh[:], op=Alu.add)
    sn = view(pn[:], 3, [[4, KG]])  # neighbor typespin

    # angular per-neighbor weights fcm = fca*m_ang; neighbor spin (ext bufs)
    fcme = kp.tile([PT, 36 * G], F32, tag="fcme")
    se = kp.tile([PT, 36 * G], F32, tag="se")
    fcm_b = view(fcme[:], 0, [[36, G], [1, K]])
    mA = kp.tile([PT, KG], F32, tag="mA")
    nc.vector.tensor_tensor(out=mA[:], in0=a1h[:], in1=m2[:], op=Alu.mult)
    mA3 = mA[:].rearrange("p (g k) -> p g k", g=G)
    gang3 = gang[:].rearrange("p (g k) -> p g k", g=G)
    nc.vector.tensor_tensor(out=fcm_b, in0=mA3, in1=gang3, op=Alu.mult)
    nc.vector.tensor_tensor(out=fcm_b, in0=fcm_b, in1=mA3, op=Alu.add)
    nc.scalar.copy(
        view(se[:], 0, [[36, G], [1, K]]), view(pn[:], 3, [[4 * K, G], [4, K]])
    )

    # wrap-around copies: ext[:, 24:36] = ext[:, 0:12]
    for e in (*ue, fcme, se):
        nc.vector.tensor_copy(
            view(e[:], K, [[36, G], [1, 12]]), view(e[:], 0, [[36, G], [1, 12]])
        )

    # x map and 2x (on ACT)
    xx = kp.tile([PT, KG], F32, tag="xx")
    nc.scalar.activation(xx[:], dd[:], Act.Copy, bias=BX_, scale=AX_)
    x2 = kp.tile([PT, KG], F32, tag="x2")
    nc.scalar.activation(x2[:], xx[:], Act.Copy, scale=2.0)

    # feature accumulator
    featt = app.tile([PT, G * FEAT], F32, tag="featt")

    def rad_reduce(src_ap, col):
        nc.vector.tensor_reduce(
            out=view(featt[:], col, [[FEAT, G]]),
            in_=view(src_ap, 0, [[K, G], [1, K]]),
            axis=AX.X,
            op=Alu.add,
        )

    # Deferred DVE reduces: pool-produced reduce inputs are reduced on DVE a
    # couple of angular orders later so DVE never head-of-line blocks on pool.
    pending = []

    def defer(tag, fn):
        pending.append((tag, fn))

    def drain_deferred(now):
        rest = []
        for tag, fn in pending:
            if tag <= now:
                fn()
            else:
                rest.append((tag, fn))
        pending[:] = rest

    # ---- radial chains: S_t = wr*T_t(x) on pool; B values are sn * S_t.
    # Emitted interleaved with the angular loop (via emit_radial_order) so
    # pool radial work fills gaps while DVE runs the angular A chain.
    rtmpb = [kp.tile([PT, KG], F32, name=f"rtmpb{i}", tag=f"rtmpb{i}") for i in range(8)]
    rbuf = [Sr0a] + [
        kp.tile([PT, KG], F32, name=f"Sr{i}a", tag=f"Sr{i}a") for i in range(1, 8)
    ]
    rtmpa = [kp.tile([PT, KG], F32, name=f"rtmpa{i}", tag=f"rtmpa{i}") for i in range(2)]

    def rad_b(src, t, slot):
        rt = rtmpb[t % 8]
        nc.gpsimd.tensor_tensor(out=rt[:], in0=src[:], in1=sn, op=Alu.mult)
        defer(slot + 2, lambda rt=rt, t=t: rad_reduce(rt[:], NRAD + t))

    def emit_radial_order(t, slot):
        if t == 0:
            defer(slot + 2, lambda: rad_reduce(Sr0a[:], 0))
            rad_b(Sr0a, 0, slot)
        elif t == 1:
            nc.gpsimd.tensor_tensor(
                out=rbuf[1][:], in0=xx[:], in1=rbuf[0][:], op=Alu.mult
            )
            defer(slot + 2, lambda: rad_reduce(rbuf[1][:], 1))
            rad_b(rbuf[1], 1, slot)
        else:
            cur, prev, dst = rbuf[(t - 1) % 8], rbuf[(t - 2) % 8], rbuf[t % 8]
            ra = rtmpa[t % 2]
            nc.gpsimd.tensor_tensor(out=ra[:], in0=x2[:], in1=cur[:], op=Alu.mult)
            nc.gpsimd.tensor_tensor(
                out=dst[:], in0=ra[:], in1=prev[:], op=Alu.subtract
            )
            defer(slot + 2, lambda dst=dst, t=t: rad_reduce(dst[:], t))
            rad_b(dst, t, slot)

    # ---- angular: cos(theta) over pair rectangle (g, d=1..12, i=0..23) ----
    ct = app.tile([PT, PAIRG], F32, tag="ct")
    tp = app.tile([PT, PAIRG], F32, tag="atmp")  # aliases atmp (disjoint lifetime)
    ct3 = view(ct[:], 0, [[PAIR, G], [K, DG], [1, K]])
    tp3 = view(tp[:], 0, [[PAIR, G], [K, DG], [1, K]])
    for c in range(3):
        jj = view(ue[c][:], 0, [[36, G], [0, DG], [1, K]])
        kk = view(ue[c][:], 1, [[36, G], [1, DG], [1, K]])
        if c == 0:
            nc.vector.tensor_tensor(out=ct3, in0=jj, in1=kk, op=Alu.mult)
        else:
            nc.vector.tensor_tensor(out=tp3, in0=jj, in1=kk, op=Alu.mult)
            nc.vector.tensor_tensor(out=ct[:], in0=ct[:], in1=tp[:], op=Alu.add)
    c2t = app.tile([PT, PAIRG], F32, tag="c2t")
    nc.scalar.activation(c2t[:], ct[:], Act.Copy, scale=2.0)

    # pair weights w = fcm_j*fcm_k (d=12 halved) and pair sign ss = s_j*s_k
    Sa0 = app.tile([PT, PAIRG], F32, tag="Sa0")
    ss = app.tile([PT, PAIRG], F32, tag="ss")
    for eng, dst, e in ((nc.vector, Sa0, fcme), (nc.gpsimd, ss, se)):
        eng.tensor_tensor(
            out=view(dst[:], 0, [[PAIR, G], [K, DG], [1, K]]),
            in0=view(e[:], 0, [[36, G], [0, DG], [1, K]]),
            in1=view(e[:], 1, [[36, G], [1, DG], [1, K]]),
            op=Alu.mult,
        )
    dv = view(Sa0[:], (DG - 1) * K, [[PAIR, G], [1, K]])
    nc.vector.tensor_scalar(out=dv, in0=dv, scalar1=0.5, scalar2=None, op0=Alu.mult)

    def ang_reduce_a(src_ap, t):
        # chain A (unweighted): full 288-wide segment reduce on DVE
        nc.vector.tensor_reduce(
            out=view(featt[:], 2 * NRAD + t, [[FEAT, G]]),
            in_=view(src_ap, 0, [[PAIR, G], [1, PAIR]]),
            axis=AX.X,
            op=Alu.add,
        )

    def ang_reduce_b(src, scratch, t):
        # chain B: even t -> gpsimd halving tree 288->9 + tiny DVE tail;
        # odd t -> plain DVE segment reduce (balances the two engines).
        # All DVE parts are deferred 2 orders (inputs come from pool).
        if t % 2 == 1:
            defer(
                t + 3,
                lambda src=src, t=t: nc.vector.tensor_reduce(
                    out=view(featt[:], 2 * NRAD + NANG + t, [[FEAT, G]]),
                    in_=view(src[:], 0, [[PAIR, G], [1, PAIR]]),
                    axis=AX.X,
                    op=Alu.add,
                ),
            )
            return
        nc.gpsimd.tensor_tensor(
            out=view(scratch[:], 0, [[PAIR, G], [1, 144]]),
            in0=view(src[:], 0, [[PAIR, G], [1, 144]]),
            in1=view(src[:], 144, [[PAIR, G], [1, 144]]),
            op=Alu.add,
        )
        n = 144
        while n > 9:
            nc.gpsimd.tensor_tensor(
                out=view(scratch[:], 0, [[PAIR, G], [1, n // 2]]),
                in0=view(scratch[:], 0, [[PAIR, G], [1, n // 2]]),
                in1=view(scratch[:], n // 2, [[PAIR, G], [1, n // 2]]),
                op=Alu.add,
            )
            n //= 2
        defer(
            t + 3,
            lambda scratch=scratch, t=t: nc.vector.tensor_reduce(
                out=view(featt[:], 2 * NRAD + NANG + t, [[FEAT, G]]),
                in_=view(scratch[:], 0, [[PAIR, G], [1, 9]]),
                axis=AX.X,
                op=Alu.add,
            ),
        )

    # chain A (weights w) on DVE; B values are ss * S_t (sign flip only)
    btree = [
        app.tile([PT, PAIRG], F32, name="btree0", tag="btree0"),
        # btree1 first written at t=2, after ct's last read (the S1 mult)
        app.tile([PT, PAIRG], F32, name="btree1", tag="ct"),
    ]
    btmp = [app.tile([PT, PAIRG], F32, name=f"btmp{i}", tag=f"btmp{i}") for i in range(3)]

    def chain_b(src, t):
        bt = btmp[t % 3]
        nc.gpsimd.tensor_tensor(out=bt[:], in0=ss[:], in1=src[:], op=Alu.mult)
        ang_reduce_b(bt, btree[(t // 2) % 2], t)

    rad_emitted = 0

    def drain_radial(n, slot):
        nonlocal rad_emitted
        for _ in range(n):
            if rad_emitted < NRAD:
                emit_radial_order(rad_emitted, slot)
                rad_emitted += 1

    drain_radial(3, 0)
    ang_reduce_a(Sa0[:], 0)
    chain_b(Sa0, 0)
    abuf = [Sa0] + [
        app.tile([PT, PAIRG], F32, name=f"Sa{i}", tag=f"Sa{i}") for i in (1, 2, 3)
    ]
    nc.vector.tensor_tensor(out=abuf[1][:], in0=ct[:], in1=abuf[0][:], op=Alu.mult)
    ang_reduce_a(abuf[1][:], 1)
    chain_b(abuf[1], 1)
    drain_radial(2, 1)
    atmp = app.tile([PT, PAIRG], F32, tag="atmp")
    for t in range(2, NANG):
        drain_deferred(t)
        cur, prev, dst = abuf[(t - 1) % 4], abuf[(t - 2) % 4], abuf[t % 4]
        nc.vector.tensor_tensor(out=atmp[:], in0=c2t[:], in1=cur[:], op=Alu.mult)
        nc.vector.tensor_tensor(
            out=dst[:], in0=atmp[:], in1=prev[:], op=Alu.subtract
        )
        ang_reduce_a(dst[:], t)
        chain_b(dst, t)
        drain_radial(2, t)
    drain_radial(NRAD, NANG)
    drain_deferred(10**9)

    # ---- store ----
    nc.sync.dma_start(
        out=feat[base : base + PT * G, :].rearrange("(p g) f -> p (g f)", p=PT),
        in_=featt[:],
    )

    if dbg is not None:
        for name, t in [
            ("pn", pn), ("dd", dd), ("rinv", rinv), ("grad", grad),
            ("gang", gang), ("xx", xx), ("ue0", ue[0]),
            ("fcme", fcme), ("ct", ct),
        ]:
            d = dbg[name]
            w = d.shape[1] // SUP
            nc.sync.dma_start(out=d[:, s * w : (s + 1) * w], in_=t[:])


DBG_SHAPES = {
    "pn": KG * 4, "dd": KG, "rinv": KG, "grad": KG, "gang": KG,
    "xx": KG, "ue0": 36 * G, "fcme": 36 * G,
    "ct": PAIRG,
}


def build_program(debug=False):
    nc = bacc.Bacc(
        "TRN2",
        target_bir_lowering=False,
        debug=False,
    )
    pos4 = nc.dram_tensor("pos4", [NPAD, ROWE], F32, kind="ExternalInput").ap()
    idx16 = nc.dram_tensor(
        "idx16", [SUP * GQ * PT, KG // GQ * PT // 16], I16, kind="ExternalInput"
    ).ap()
    pself = nc.dram_tensor("pself", [NPC, 4], F32, kind="ExternalInput").ap()
    feat = nc.dram_tensor("feat", [NPC, FEAT], F32, kind="ExternalOutput").ap()
    dbg = None
    if debug:
        dbg = {
            name: nc.dram_tensor(
                f"dbg_{name}", [PT, SUP * w], F32, kind="ExternalOutput"
            ).ap()
            for name, w in DBG_SHAPES.items()
        }
    with tile.TileContext(nc) as tc, ExitStack() as ctx:
        io = ctx.enter_context(tc.tile_pool(name="io", bufs=2))
        kp = ctx.enter_context(tc.tile_pool(name="kspace", bufs=1))
        app = ctx.enter_context(tc.tile_pool(name="pairspace", bufs=1))
        const = ctx.enter_context(tc.tile_pool(name="const", bufs=1))
        half_pi = const.tile([PT, 1], F32, tag="half_pi")
        nc.gpsimd.memset(half_pi[:], HALF_PI)
        for s in range(SUP):
            build_supertile(
                nc, io, kp, app, s, pos4, idx16, pself, feat, half_pi, dbg=dbg
            )
    nc.compile()
    return nc


_NC_CACHE = None


def get_program():
    global _NC_CACHE
    if _NC_CACHE is None:
        _NC_CACHE = build_program()
    return _NC_CACHE


def make_in_maps(positions, species_idx, neighbor_idx):
    pos4 = np.zeros((NPAD, ROWE), np.float32)
    pos4[:N, :3] = positions
    pos4[:N, 3] = 2.0 * species_idx.astype(np.float32) - 1.0  # TYPESPIN[-1, 1]
    nbrK = np.zeros((NPAD, K), np.int32)
    nbrK[:N] = neighbor_idx.reshape(N, K)

    CQ = KG // GQ
    c_idx = np.arange(KG)
    g_of, k_of = c_idx // K, c_idx % K
    p = np.arange(PT)
    in_maps = []
    for c in range(NCORES):
        cb = c * NPC
        # idx value for (supertile s, chunk cidx, partition p):
        #   nbrK[cb + s*PT*G + p*G + g(cidx), k(cidx)]
        blocks = []
        for s in range(SUP):
            atoms = cb + s * PT * G + p[None, :] * G + g_of[:, None]  # [KG, PT]
            vals = nbrK[atoms, k_of[:, None]].astype(np.int16)  # [KG, PT]
            for q in range(GQ):
                flat = vals[q * CQ : (q + 1) * CQ, :].reshape(-1)  # i = cc*128+p
                wrapped = flat.reshape(-1, 16).T  # [16, NIDX/16]
                blocks.append(np.tile(wrapped, (PT // 16, 1)))
        idx16 = np.concatenate(blocks, axis=0)  # [SUP*GQ*PT, NIDX/16]
        in_maps.append(
            {
                "pos4": pos4,
                "idx16": np.ascontiguousarray(idx16),
                "pself": np.ascontiguousarray(pos4[cb : cb + NPC, :4]),
            }
        )
    return in_maps


def run(positions, species_idx, neighbor_idx, trace=False, trace_cores=None):
    nc = get_program()
    in_maps = make_in_maps(positions, species_idx, neighbor_idx)
    res = run_bass_kernel_spmd(
        nc,
        in_maps,
        core_ids=list(range(NCORES)),
        trace=trace,
        trace_cores=trace_cores,
    )
    out = np.concatenate([res.results[c]["feat"] for c in range(NCORES)], axis=0)
    return out[:N], res


def kernel(positions, species_idx, neighbor_idx):
    out, _ = run(positions, species_idx, neighbor_idx, trace=False)
    return out



# revision 16
# speedup vs baseline: 1.2790x; 1.2790x over previous
"""Trainium2 Bass kernel for the Chebyshev atomic descriptor (gnn_message_passing).

Contract: kernel(**inputs) takes FULL unsharded inputs (positions [20000,3] f32,
species_idx [20000] i32, neighbor_idx [480000] i32) and returns the full
[20000, 52] f32 feature array. Internally shards atoms across 8 NeuronCores
(data-parallel over N) and gathers neighbor rows on-device via indirect DMA.

Algorithm: the angular (triplet) features are computed via the spherical-
harmonic addition theorem instead of the O(K^2) pair sum:
  sum_{j<k} w_j w_k T_t(u_j.u_k) = 1/2 (sum_l lam_{t,l} Q_l - F2),
  Q_l = sum_m gamma_lm B_lm^2,  B_lm = sum_j w_j Ybar_lm(u_j),  F2 = sum_j w_j^2
with real solid harmonics evaluated per neighbor by sectoral (x+iy)^m and
associated-Legendre z-ladder recurrences in fp16. All reductions over the K=24
neighbors (angular moments, radial Chebyshev chains, F2) run on the otherwise
idle TensorEngine as identity-stationary matmuls accumulating in PSUM.
"""

import math
from contextlib import ExitStack

import numpy as np

import bass_rust
import concourse.bass as bass
import concourse.bacc as bacc
import concourse.tile as tile
from concourse import mybir
from concourse.bass_utils import run_bass_kernel_spmd

F32 = mybir.dt.float32
F16 = mybir.dt.float16
I32 = mybir.dt.int32
Alu = mybir.AluOpType
Act = mybir.ActivationFunctionType
AX = mybir.AxisListType

# ---- problem constants (hardcoded per harness contract) ----
N = 20000
K = 24
NCORES = 8
NPAD = 20480
NPC = NPAD // NCORES     # atoms per core = 2560
PT = 128                 # partitions
G = 5                    # atoms per partition per supertile
SUP = NPC // (PT * G)    # supertiles per core = 4
STA = PT * G             # atoms per supertile = 640
SLOT = G * K             # neighbor slots per partition per supertile = 120
RAD_ORDER = 16
ANG_ORDER = 8
L = ANG_ORDER
NRAD = RAD_ORDER + 1     # 17
NANG = ANG_ORDER + 1     # 9
RAD_CUT = 8.0
ANG_CUT = 6.5
MIN_CUT = 0.55
FEAT = 52
NC_RECT = 9 * 9 * 2      # 162 rect comps (l, m, trig)
NRADC = NRAD + 1         # radial comps + F2 slot = 18

HALF_PI = math.pi / 2.0
AX_ = 2.0 / (RAD_CUT - MIN_CUT)
BX_ = -2.0 * MIN_CUT / (RAD_CUT - MIN_CUT) - 1.0

ROWE = 64                # gather table row: 64 f32 = 256B (dma_gather granularity)
GQ = 2                   # dma_gather calls per supertile
CQ = SLOT // GQ          # gathered slots per partition per call = 60
NIDX = CQ * PT           # indices per gather call = 7680


# ---------------------------------------------------------------------------
# host-side constant tables (ladder recurrence + quadratic-form weights)
# ---------------------------------------------------------------------------
def _dfact(n):
    r = 1
    while n > 1:
        r *= n
        n -= 2
    return r


def _a_norm(l, m):
    if m == 0:
        return 1.0
    return math.sqrt(2.0 * math.factorial(l - m) / math.factorial(l + m))


def _ladder_coeffs():
    """Monic z-ladder: A~_m = 1, A~_{m+1} = z, A~_l = z A~_{l-1} + gt A~_{l-2};
    Ybar_lm = sig_lm * A~_lm * trig_m. Returns gt[(l,m)], sig[(l,m)]."""
    gt, sig = {}, {}
    for m in range(L + 1):
        k = {m: 1.0 / _dfact(2 * m - 1)}
        if m + 1 <= L:
            k[m + 1] = k[m] / (2 * m + 1)
        for l in range(m + 2, L + 1):
            beta = (2 * l - 1) / (l - m)
            gam = -(l + m - 1) / (l - m)
            k[l] = k[l - 1] / beta
            gt[(l, m)] = gam * k[l] / k[l - 2]
        for l in range(m, L + 1):
            sig[(l, m)] = _a_norm(l, m) / k[l]
    return gt, sig


def _cheb_to_legendre():
    from numpy.polynomial import legendre as npleg, chebyshev as npcheb

    lam = np.zeros((NANG, L + 1))
    for t in range(NANG):
        c = np.zeros(t + 1)
        c[t] = 1.0
        lam[t, : t + 1] = npleg.poly2leg(npcheb.cheb2poly(c))[: t + 1]
    return lam


LAM = _cheb_to_legendre()
GT, SIG = _ladder_coeffs()


def _const_tables():
    # ccoef f16 [81]: gt at slot l*9+m (l-major), 0 elsewhere
    ccoef = np.zeros(81, np.float16)
    for (l, m), v in GT.items():
        ccoef[l * 9 + m] = np.float16(v)
    # gam f32 [162]: sig^2 at rect slot (l*9+m)*2+t for valid (m<=l), else 0
    gam = np.zeros(NC_RECT, np.float32)
    for l in range(L + 1):
        for m in range(l + 1):
            g = np.float32(SIG[(l, m)]) ** 2
            gam[(l * 9 + m) * 2 + 0] = g
            if m >= 1:
                gam[(l * 9 + m) * 2 + 1] = g
    ident = np.eye(PT, dtype=np.float16)
    return ccoef, gam, ident


def view(ap, off, dims):
    """Free-dim view of a tile AP: keep the partition entry, replace free dims
    with explicit [step, count] pairs, shift the element offset by `off`."""
    base = list(ap.ap[0])
    return bass_rust.AP(ap.tensor, ap.offset + off, [base] + [list(d) for d in dims])


def build_supertile(nc, ctx, s, tl, pself, feat_dram):
    """Emit one supertile's compute. tl = dict of persistent tiles."""
    base = s * STA

    pn = tl[f"pn{s % 2}"]
    ps = tl[f"ps{s % 2}"]

    # ---- prep (f32) ----
    r_c = []
    for c in range(3):
        r = tl[f"r{c}"]
        nc.vector.tensor_tensor(
            out=r[:],
            in0=view(pn[:], c, [[4, SLOT]]),
            in1=view(ps[:], c, [[4, G], [0, K]]),
            op=Alu.subtract,
        )
        r_c.append(r)
    sq = []
    for c in range(3):
        q = tl[f"sq{c}"]
        nc.scalar.activation(q[:], r_c[c][:], Act.Square)
        sq.append(q)
    d2 = tl["d2"]
    nc.vector.tensor_tensor(out=d2[:], in0=sq[0][:], in1=sq[1][:], op=Alu.add)
    nc.vector.tensor_tensor(out=d2[:], in0=d2[:], in1=sq[2][:], op=Alu.add)
    nc.vector.tensor_scalar_max(d2[:], d2[:], 1e-18)
    dd = tl["dd"]
    nc.scalar.sqrt(dd[:], d2[:])
    rinv = tl["rinv"]
    nc.vector.reciprocal(rinv[:], dd[:])

    # unit vector: x,y go straight into SEC block m=1; z separate (f16)
    SEC = tl["SEC"]
    nc.vector.tensor_tensor(
        out=view(SEC[:], 0, [[1, SLOT]]), in0=r_c[0][:], in1=rinv[:], op=Alu.mult
    )
    nc.vector.tensor_tensor(
        out=view(SEC[:], SLOT, [[1, SLOT]]), in0=r_c[1][:], in1=rinv[:], op=Alu.mult
    )
    uz = tl["uz"]
    nc.vector.tensor_tensor(out=uz[:], in0=r_c[2][:], in1=rinv[:], op=Alu.mult)

    m2 = tl["m2"]
    nc.vector.tensor_scalar(
        out=m2[:], in0=dd[:], scalar1=MIN_CUT, scalar2=None, op0=Alu.is_gt
    )
    half_pi = tl["half_pi"]

    # radial weight wr = fc * mask  (wr = mh*(grad+1), mh = 0.5*mask)
    dcr = tl["dcr"]
    nc.vector.tensor_scalar_min(dcr[:], dd[:], RAD_CUT)
    grad = tl["grad"]
    nc.scalar.activation(
        grad[:], dcr[:], Act.Sin, bias=half_pi[:], scale=-math.pi / RAD_CUT
    )
    m1h = tl["m1h"]
    nc.vector.tensor_scalar(
        out=m1h[:], in0=dd[:], scalar1=RAD_CUT, scalar2=0.5, op0=Alu.is_le, op1=Alu.mult
    )
    mh = tl["mh"]
    nc.vector.tensor_tensor(out=mh[:], in0=m1h[:], in1=m2[:], op=Alu.mult)
    Srad = tl["Srad"]
    # S0 = wr (f16), written directly into the radial chain tile
    nc.vector.scalar_tensor_tensor(
        out=view(Srad[:], 0, [[1, SLOT]]),
        in0=grad[:],
        scalar=1.0,
        in1=mh[:],
        op0=Alu.add,
        op1=Alu.mult,
    )

    # angular weight w = fca * mask
    dca = tl["dca"]
    nc.vector.tensor_scalar_min(dca[:], dd[:], ANG_CUT)
    gang = tl["gang"]
    nc.scalar.activation(
        gang[:], dca[:], Act.Sin, bias=half_pi[:], scale=-math.pi / ANG_CUT
    )
    a1h = tl["a1h"]
    nc.vector.tensor_scalar(
        out=a1h[:], in0=dd[:], scalar1=ANG_CUT, scalar2=0.5, op0=Alu.is_le, op1=Alu.mult
    )
    mA = tl["mA"]
    nc.vector.tensor_tensor(out=mA[:], in0=a1h[:], in1=m2[:], op=Alu.mult)
    wh = tl["wh"]
    nc.vector.scalar_tensor_tensor(
        out=wh[:], in0=gang[:], scalar=1.0, in1=mA[:], op0=Alu.add, op1=Alu.mult
    )
    snh = tl["snh"]
    nc.scalar.copy(snh[:], view(pn[:], 3, [[4, SLOT]]))
    wsh = tl["wsh"]
    nc.vector.tensor_tensor(out=wsh[:], in0=wh[:], in1=snh[:], op=Alu.mult)

    # radial chebyshev argument (f16)
    xxh = tl["xxh"]
    nc.scalar.activation(xxh[:], dd[:], Act.Copy, bias=BX_, scale=AX_)
    x2h = tl["x2h"]
    nc.scalar.activation(x2h[:], dd[:], Act.Copy, bias=2 * BX_, scale=2 * AX_)

    # ---- radial chain (f16): S_t into Srad slots, B chain = S*spin ----
    nc.vector.tensor_tensor(
        out=view(Srad[:], SLOT, [[1, SLOT]]),
        in0=xxh[:],
        in1=view(Srad[:], 0, [[1, SLOT]]),
        op=Alu.mult,
    )
    rtmp = tl["rtmp"]
    for t in range(2, NRAD):
        nc.vector.tensor_tensor(
            out=rtmp[:],
            in0=x2h[:],
            in1=view(Srad[:], (t - 1) * SLOT, [[1, SLOT]]),
            op=Alu.mult,
        )
        nc.vector.tensor_tensor(
            out=view(Srad[:], t * SLOT, [[1, SLOT]]),
            in0=rtmp[:],
            in1=view(Srad[:], (t - 2) * SLOT, [[1, SLOT]]),
            op=Alu.subtract,
        )
    # F2 = w^2 appended as radial comp 17 (chain A)
    nc.vector.tensor_tensor(
        out=view(Srad[:], NRAD * SLOT, [[1, SLOT]]), in0=wh[:], in1=wh[:], op=Alu.mult
    )
    # chain B: spin-weighted radial values, appended as comps 18..34 of Srad
    nc.vector.tensor_tensor(
        out=view(Srad[:], NRADC * SLOT, [[SLOT, NRAD], [1, SLOT]]),
        in0=view(Srad[:], 0, [[SLOT, NRAD], [1, SLOT]]),
        in1=view(snh[:], 0, [[0, NRAD], [1, SLOT]]),
        op=Alu.mult,
    )

    # ---- sectoral recurrence on Pool (f16): c_m,s_m for m=2..8 ----
    tc_ = tl["tc_"]
    td_ = tl["td_"]
    for m in range(2, L + 1):
        cp = (m - 2) * 2 * SLOT      # c_{m-1}
        sp = cp + SLOT               # s_{m-1}
        cm = (m - 1) * 2 * SLOT
        sm = cm + SLOT
        ux_v = view(SEC[:], 0, [[1, SLOT]])
        uy_v = view(SEC[:], SLOT, [[1, SLOT]])
        nc.gpsimd.tensor_tensor(out=tc_[:], in0=ux_v, in1=view(SEC[:], cp, [[1, SLOT]]), op=Alu.mult)
        nc.gpsimd.tensor_tensor(out=td_[:], in0=uy_v, in1=view(SEC[:], sp, [[1, SLOT]]), op=Alu.mult)
        nc.gpsimd.tensor_tensor(
            out=view(SEC[:], cm, [[1, SLOT]]), in0=tc_[:], in1=td_[:], op=Alu.subtract
        )
        nc.gpsimd.tensor_tensor(out=tc_[:], in0=ux_v, in1=view(SEC[:], sp, [[1, SLOT]]), op=Alu.mult)
        nc.gpsimd.tensor_tensor(out=td_[:], in0=uy_v, in1=view(SEC[:], cp, [[1, SLOT]]), op=Alu.mult)
        nc.gpsimd.tensor_tensor(
            out=view(SEC[:], sm, [[1, SLOT]]), in0=tc_[:], in1=td_[:], op=Alu.add
        )

    # ---- z-ladder (f16, l-major LAD: slot (l*9+m)*SLOT) ----
    LAD = tl["LAD"]
    ccoef = tl["ccoef"]
    # l = m+1 diagonal row: A~_{m+1,m} = z for m=0..7 (slots m*10+9)
    nc.vector.tensor_copy(
        out=view(LAD[:], 9 * SLOT, [[10 * SLOT, 8], [1, SLOT]]),
        in_=view(uz[:], 0, [[0, 8], [1, SLOT]]),
    )
    lt = tl["lt"]
    for l in range(2, L + 1):
        nm = l - 1  # m = 0..l-2
        nc.vector.tensor_tensor(
            out=view(lt[:], 0, [[SLOT, nm], [1, SLOT]]),
            in0=view(uz[:], 0, [[0, nm], [1, SLOT]]),
            in1=view(LAD[:], (l - 1) * 9 * SLOT, [[SLOT, nm], [1, SLOT]]),
            op=Alu.mult,
        )
        nc.vector.tensor_tensor(
            out=view(lt[:], 7 * SLOT, [[SLOT, nm], [1, SLOT]]),
            in0=view(ccoef[:], l * 9, [[1, nm], [0, SLOT]]),
            in1=view(LAD[:], (l - 2) * 9 * SLOT, [[SLOT, nm], [1, SLOT]]),
            op=Alu.mult,
        )
        nc.vector.tensor_tensor(
            out=view(LAD[:], l * 9 * SLOT, [[SLOT, nm], [1, SLOT]]),
            in0=view(lt[:], 0, [[SLOT, nm], [1, SLOT]]),
            in1=view(lt[:], 7 * SLOT, [[SLOT, nm], [1, SLOT]]),
            op=Alu.add,
        )

    # ---- weight tiles WA/WB (f16): (m, trig) slots ----
    WA, WB = tl["WA"], tl["WB"]
    nc.vector.tensor_copy(out=view(WA[:], 0, [[1, SLOT]]), in_=wh[:])
    nc.vector.tensor_copy(out=view(WB[:], 0, [[1, SLOT]]), in_=wsh[:])
    for m in range(1, L + 1):
        sec_b = view(SEC[:], (m - 1) * 2 * SLOT, [[SLOT, 2], [1, SLOT]])
        nc.vector.tensor_tensor(
            out=view(WA[:], m * 2 * SLOT, [[SLOT, 2], [1, SLOT]]),
            in0=view(wh[:], 0, [[0, 2], [1, SLOT]]),
            in1=sec_b,
            op=Alu.mult,
        )
        nc.vector.tensor_tensor(
            out=view(WB[:], m * 2 * SLOT, [[SLOT, 2], [1, SLOT]]),
            in0=view(wsh[:], 0, [[0, 2], [1, SLOT]]),
            in1=sec_b,
            op=Alu.mult,
        )

    # ---- products into MP rect (f16): MP[(l*9+m)*2+t] = W[m,t] * A~[l,m] ----
    for chain, (W, MP) in enumerate(((WA, tl["MPA"]), (WB, tl["MPB"]))):
        for m in range(L + 1):
            nl = 9 - m
            nc.vector.tensor_tensor(
                out=view(MP[:], m * 20 * SLOT, [[18 * SLOT, nl], [SLOT, 2], [1, SLOT]]),
                in0=view(W[:], m * 2 * SLOT, [[0, nl], [SLOT, 2], [1, SLOT]]),
                in1=view(LAD[:], m * 10 * SLOT, [[9 * SLOT, nl], [0, 2], [1, SLOT]]),
                op=Alu.mult,
            )

    # ---- K-reduction on PE: identity-stationary accumulating matmuls ----
    # out per matmul must stay inside one PSUM bank (512 f32): split the 162
    # rect comps into two 81-comp groups per chain at bank-aligned offsets.
    ident = tl["ident"]
    accA, accB, accR = tl["accA"], tl["accB"], tl["accR"]
    HC = NC_RECT // 2  # 81
    for acc, MP in ((accA, tl["MPA"]), (accB, tl["MPB"])):
        for half in range(2):
            for k in range(K):
                nc.tensor.matmul(
                    view(acc[:], half * 512, [[1, HC * G]]),
                    ident[:],
                    view(MP[:], half * HC * SLOT + k, [[SLOT, HC], [K, G]]),
                    start=(k == 0),
                    stop=(k == K - 1),
                )
    for k in range(K):
        nc.tensor.matmul(
            view(accR[:], 0, [[1, (NRADC + NRAD) * G]]),
            ident[:],
            view(Srad[:], k, [[SLOT, NRADC + NRAD], [K, G]]),
            start=(k == 0),
            stop=(k == K - 1),
        )

    # ---- evacuate: squares of angular moments; radial/F2 copies ----
    SQ = tl["SQ"]
    soff = s * 2 * NC_RECT * G
    for ci, acc in enumerate((accA, accB)):
        for half in range(2):
            nc.scalar.activation(
                view(SQ[:], soff + ci * NC_RECT * G + half * HC * G, [[1, HC * G]]),
                view(acc[:], half * 512, [[1, HC * G]]),
                Act.Square,
            )
    featt = tl["featt"]
    foff = s * G * FEAT
    # rad_un (f 0..16) and rad_w (f 17..33): iter (t, g) -> featt[g*52 + f]
    nc.scalar.copy(
        out=view(featt[:], foff + 0, [[1, NRAD], [FEAT, G]]),
        in_=view(accR[:], 0, [[G, NRAD], [1, G]]),
    )
    nc.scalar.copy(
        out=view(featt[:], foff + NRAD, [[1, NRAD], [FEAT, G]]),
        in_=view(accR[:], NRADC * G, [[G, NRAD], [1, G]]),
    )
    # F2 (radial comp 17 of chain A) -> F2S[s]
    nc.scalar.copy(
        out=view(tl["F2S"][:], s * G, [[1, G]]),
        in_=view(accR[:], NRAD * G, [[1, G]]),
    )


def build_epilogue(nc, tl, feat_dram):
    """gamma-weight + Q-reduce + lambda-mix for all supertiles, then store."""
    SQ, gam = tl["SQ"], tl["gam"]
    # QT = gam * SQ (in place), per (chain, supertile)
    for cs in range(2 * SUP):
        o = cs * NC_RECT * G
        nc.vector.tensor_tensor(
            out=view(SQ[:], o, [[1, NC_RECT * G]]),
            in0=view(SQ[:], o, [[1, NC_RECT * G]]),
            in1=view(gam[:], 0, [[1, NC_RECT], [0, G]]),
            op=Alu.mult,
        )
    # Q[l] = sum over (m,trig): rect layout (l,m,t,g): l stride 18G, m 2G, t G
    Q = tl["Q"]  # [PT, SUP*2*9*G] layout (s, chain, l, g)
    for s in range(SUP):
        for chain in range(2):
            o = (s * 2 + chain) * NC_RECT * G
            nc.vector.tensor_reduce(
                out=view(Q[:], (s * 2 + chain) * 9 * G, [[G, 9], [1, G]]),
                in_=view(SQ[:], o, [[18 * G, 9], [1, G], [G, 18]]),
                axis=AX.X,
                op=Alu.add,
            )
    # F2h = 0.5 * F2
    F2S, F2h = tl["F2S"], tl["F2h"]
    nc.vector.tensor_scalar(
        out=F2h[:], in0=F2S[:], scalar1=0.5, scalar2=None, op0=Alu.mult
    )
    # ang[t] = sum_l 0.5*lam[t,l] Q_l - 0.5*F2, written into featt cols
    featt = tl["featt"]
    mixa, mixb = tl["mixa"], tl["mixb"]
    for chain in range(2):
        fbase = 2 * NRAD + chain * NANG
        for t in range(NANG):
            ls = [l for l in range(t % 2, t + 1, 2)]
            acc = None
            for i, l in enumerate(ls):
                qv = view(Q[:], (chain * 9 + l) * G, [[2 * 9 * G, SUP], [1, G]])
                lam = 0.5 * float(LAM[t, l])
                last = i == len(ls) - 1
                dst = (
                    view(featt[:], fbase + t, [[G * FEAT, SUP], [FEAT, G]])
                    if last
                    else view((mixb if acc is mixa else mixa)[:], 0, [[G, SUP], [1, G]])
                )
                if i == 0:
                    # (Q*lam) - F2h
                    nc.vector.scalar_tensor_tensor(
                        out=dst,
                        in0=qv,
                        scalar=lam,
                        in1=view(F2h[:], 0, [[G, SUP], [1, G]]),
                        op0=Alu.mult,
                        op1=Alu.subtract,
                    )
                else:
                    src = view((mixa if acc is mixa else mixb)[:], 0, [[G, SUP], [1, G]])
                    nc.vector.scalar_tensor_tensor(
                        out=dst, in0=qv, scalar=lam, in1=src, op0=Alu.mult, op1=Alu.add
                    )
                acc = mixa if (acc is not mixa) else mixb
    # store per supertile
    for s in range(SUP):
        nc.sync.dma_start(
            out=feat_dram[s * STA : (s + 1) * STA, :].rearrange(
                "(p g) f -> p (g f)", p=PT
            ),
            in_=view(featt[:], s * G * FEAT, [[1, G * FEAT]]),
        )


def build_program():
    I16 = mybir.dt.int16
    nc = bacc.Bacc("TRN2", target_bir_lowering=False, debug=False)
    pos4 = nc.dram_tensor("pos4", [NPAD, ROWE], F32, kind="ExternalInput").ap()
    idx = nc.dram_tensor(
        "idx", [SUP * GQ * PT, NIDX // 16], I16, kind="ExternalInput"
    ).ap()
    pself = nc.dram_tensor("pself", [NPC, 4], F32, kind="ExternalInput").ap()
    ident_d = nc.dram_tensor("ident", [PT, PT], F16, kind="ExternalInput").ap()
    ccoef_d = nc.dram_tensor("ccoef", [PT, 81], F16, kind="ExternalInput").ap()
    gam_d = nc.dram_tensor("gam", [PT, NC_RECT], F32, kind="ExternalInput").ap()
    feat = nc.dram_tensor("feat", [NPC, FEAT], F32, kind="ExternalOutput").ap()

    with tile.TileContext(nc) as tc, ExitStack() as ctx:
        const = ctx.enter_context(tc.tile_pool(name="const", bufs=1))
        io = ctx.enter_context(tc.tile_pool(name="io", bufs=1))
        kp = ctx.enter_context(tc.tile_pool(name="kspace", bufs=1))
        psum = ctx.enter_context(tc.tile_pool(name="psum", bufs=1, space="PSUM"))

        tl = {}

        def T(pool, name, shape, dtype):
            tl[name] = pool.tile(shape, dtype, name=name, tag=name)
            return tl[name]

        # constants
        T(const, "ident", [PT, PT], F16)
        T(const, "ccoef", [PT, 81], F16)
        T(const, "gam", [PT, NC_RECT], F32)
        T(const, "half_pi", [PT, 1], F32)
        nc.sync.dma_start(out=tl["ident"][:], in_=ident_d)
        nc.sync.dma_start(out=tl["ccoef"][:], in_=ccoef_d)
        nc.sync.dma_start(out=tl["gam"][:], in_=gam_d)
        nc.gpsimd.memset(tl["half_pi"][:], HALF_PI)

        # io (double-buffered via explicit 0/1 tiles)
        I16 = mybir.dt.int16
        for b in range(2):
            T(io, f"pn{b}", [PT, SLOT * 4], F32)
            T(io, f"ps{b}", [PT, G * 4], F32)
            T(io, f"pnw{b}", [PT, CQ * ROWE], F32)
            T(io, f"idxt{b}", [PT, NIDX // 16], I16)

        # prep f32
        for nm in ("r0", "r1", "r2", "sq0", "sq1", "sq2", "d2", "dd", "rinv",
                   "m2", "dcr", "grad", "m1h", "mh", "dca", "gang", "a1h", "mA"):
            T(kp, nm, [PT, SLOT], F32)
        # f16 working set
        for nm in ("uz", "wh", "snh", "wsh", "xxh", "x2h", "rtmp", "tc_", "td_"):
            T(kp, nm, [PT, SLOT], F16)
        T(kp, "SEC", [PT, 8 * 2 * SLOT], F16)
        T(kp, "LAD", [PT, 81 * SLOT], F16)
        T(kp, "lt", [PT, 14 * SLOT], F16)
        T(kp, "WA", [PT, NC_RECT // 9 * SLOT], F16)
        T(kp, "WB", [PT, NC_RECT // 9 * SLOT], F16)
        T(kp, "MPA", [PT, NC_RECT * SLOT], F16)
        T(kp, "MPB", [PT, NC_RECT * SLOT], F16)
        T(kp, "Srad", [PT, (NRADC + NRAD) * SLOT], F16)
        T(kp, "SQ", [PT, SUP * 2 * NC_RECT * G], F32)
        T(kp, "featt", [PT, SUP * G * FEAT], F32)
        T(kp, "F2S", [PT, SUP * G], F32)
        T(kp, "F2h", [PT, SUP * G], F32)
        T(kp, "Q", [PT, SUP * 2 * 9 * G], F32)
        T(kp, "mixa", [PT, SUP * G], F32)
        T(kp, "mixb", [PT, SUP * G], F32)

        # psum accumulators (bank-padded: each matmul target inside one bank)
        T(psum, "accA", [PT, 1024], F32)
        T(psum, "accB", [PT, 1024], F32)
        T(psum, "accR", [PT, 512], F32)

        # one-time: zero invalid MP slots (m > l) and LAD diag seeds = 1
        for MPn in ("MPA", "MPB"):
            MP = tl[MPn]
            for m in range(1, L + 1):
                nc.gpsimd.memset(
                    view(MP[:], m * 2 * SLOT, [[18 * SLOT, m], [1, 2 * SLOT]]), 0.0
                )
        nc.gpsimd.memset(
            view(tl["LAD"][:], 0, [[10 * SLOT, 9], [1, SLOT]]), 1.0
        )
        # m=0 sin-weight slots stay zero (kills the nonexistent m=0 sin comps)
        nc.gpsimd.memset(view(tl["WA"][:], SLOT, [[1, SLOT]]), 0.0)
        nc.gpsimd.memset(view(tl["WB"][:], SLOT, [[1, SLOT]]), 0.0)

        def gather(s):
            b = s % 2
            pn = tl[f"pn{b}"]
            for q in range(GQ):
                qb = (s * GQ + q) % 2
                idxt = tl[f"idxt{qb}"]
                pnw = tl[f"pnw{qb}"]
                row0 = (s * GQ + q) * PT
                nc.sync.dma_start(out=idxt[:], in_=idx[row0 : row0 + PT, :])
                nc.gpsimd.dma_gather(
                    out_ap=view(pnw[:], 0, [[ROWE, CQ], [1, ROWE]]),
                    in_ap=pos4,
                    idxs_ap=idxt[:],
                    num_idxs=NIDX,
                    num_idxs_reg=NIDX,
                    elem_size=ROWE,
                    single_packet=False,
                )
                nc.scalar.copy(
                    out=view(pn[:], q * CQ * 4, [[1, CQ * 4]]),
                    in_=view(pnw[:], 0, [[ROWE, CQ], [1, 4]]),
                )
            nc.sync.dma_start(
                out=tl[f"ps{b}"][:],
                in_=pself[s * STA : (s + 1) * STA, :].rearrange(
                    "(p g) c -> p (g c)", p=PT
                ),
            )

        gather(0)
        for s in range(SUP):
            if s + 1 < SUP:
                gather(s + 1)
            build_supertile(nc, ctx, s, tl, pself, feat)
        build_epilogue(nc, tl, feat)

    nc.compile()
    return nc


_NC_CACHE = None


def get_program():
    global _NC_CACHE
    if _NC_CACHE is None:
        _NC_CACHE = build_program()
    return _NC_CACHE


def make_in_maps(positions, species_idx, neighbor_idx):
    pos4 = np.zeros((NPAD, ROWE), np.float32)
    pos4[:N, :3] = positions
    pos4[:N, 3] = 2.0 * species_idx.astype(np.float32) - 1.0
    nbrK = np.zeros((NPAD, K), np.int32)
    nbrK[:N] = neighbor_idx.reshape(N, K)

    ccoef, gam, ident = _const_tables()
    ccoef_t = np.broadcast_to(ccoef, (PT, 81)).copy()
    gam_t = np.broadcast_to(gam, (PT, NC_RECT)).copy()

    c_idx = np.arange(SLOT)
    g_of, k_of = c_idx // K, c_idx % K
    p = np.arange(PT)
    in_maps = []
    for c in range(NCORES):
        cb = c * NPC
        blocks = []
        for s in range(SUP):
            # vals[slot, p] = nbrK[cb + s*STA + p*G + g(slot), k(slot)]
            atoms = cb + s * STA + p[None, :] * G + g_of[:, None]  # [SLOT, PT]
            vals = nbrK[atoms, k_of[:, None]].astype(np.int16)
            for q in range(GQ):
                flat = vals[q * CQ : (q + 1) * CQ, :].reshape(-1)  # i = cc*128+p
                wrapped = flat.reshape(-1, 16).T  # [16, NIDX/16]
                blocks.append(np.tile(wrapped, (PT // 16, 1)))
        idx16 = np.concatenate(blocks, axis=0)  # [SUP*GQ*PT, NIDX/16]
        in_maps.append(
            {
                "pos4": pos4,
                "idx": np.ascontiguousarray(idx16),
                "pself": np.ascontiguousarray(pos4[cb : cb + NPC, :4]),
                "ident": ident,
                "ccoef": ccoef_t,
                "gam": gam_t,
            }
        )
    return in_maps


def run(positions, species_idx, neighbor_idx, trace=False, trace_cores=None):
    nc = get_program()
    in_maps = make_in_maps(positions, species_idx, neighbor_idx)
    res = run_bass_kernel_spmd(
        nc,
        in_maps,
        core_ids=list(range(NCORES)),
        trace=trace,
        trace_cores=trace_cores,
    )
    out = np.concatenate([res.results[c]["feat"] for c in range(NCORES)], axis=0)
    return out[:N], res


def kernel(positions, species_idx, neighbor_idx):
    out, _ = run(positions, species_idx, neighbor_idx, trace=False)
    return out


# revision 55
# speedup vs baseline: 1.5409x; 1.2048x over previous
"""Trainium2 Bass kernel for the Chebyshev atomic descriptor (gnn_message_passing).

Contract: kernel(**inputs) takes FULL unsharded inputs (positions [20000,3] f32,
species_idx [20000] i32, neighbor_idx [480000] i32) and returns the full
[20000, 52] f32 feature array. Internally shards atoms across 8 NeuronCores
(data-parallel over N) and gathers neighbor rows on-device via indirect DMA.

Algorithm: the angular (triplet) features are computed via the spherical-
harmonic addition theorem instead of the O(K^2) pair sum:
  sum_{j<k} w_j w_k T_t(u_j.u_k) = 1/2 (sum_l lam_{t,l} Q_l - F2),
  Q_l = sum_m gamma_lm B_lm^2,  B_lm = sum_j w_j Ybar_lm(u_j),  F2 = sum_j w_j^2
with real solid harmonics evaluated per neighbor by sectoral (x+iy)^m and
associated-Legendre z-ladder recurrences in fp16. All reductions over the K=24
neighbors (angular moments, radial Chebyshev chains, F2) run on the otherwise
idle TensorEngine as identity-stationary matmuls accumulating in PSUM.
"""

import math
from contextlib import ExitStack

import numpy as np

import bass_rust
import concourse.bass as bass
import concourse.bacc as bacc
import concourse.tile as tile
from concourse import mybir
from concourse.bass_utils import run_bass_kernel_spmd

F32 = mybir.dt.float32
F16 = mybir.dt.float16
I32 = mybir.dt.int32
Alu = mybir.AluOpType
Act = mybir.ActivationFunctionType
AX = mybir.AxisListType

# ---- problem constants (hardcoded per harness contract) ----
N = 20000
K = 24
NCORES = 8
NPAD = 20480
NPC = NPAD // NCORES     # atoms per core = 2560
PT = 128                 # partitions
G = 5                    # atoms per partition per supertile
SUP = NPC // (PT * G)    # supertiles per core = 4
STA = PT * G             # atoms per supertile = 640
SLOT = G * K             # neighbor slots per partition per supertile = 120
RAD_ORDER = 16
ANG_ORDER = 8
L = ANG_ORDER
NRAD = RAD_ORDER + 1     # 17
NANG = ANG_ORDER + 1     # 9
RAD_CUT = 8.0
ANG_CUT = 6.5
MIN_CUT = 0.55
FEAT = 52
NC_RECT = 9 * 9 * 2      # 162 rect comps (l, m, trig)
NRADC = NRAD + 1         # radial comps + F2 slot = 18

HALF_PI = math.pi / 2.0
AX_ = 2.0 / (RAD_CUT - MIN_CUT)
BX_ = -2.0 * MIN_CUT / (RAD_CUT - MIN_CUT) - 1.0

ROWE = 64                # gather table row: 64 f32 = 256B (dma_gather granularity)
GQ = 2                   # dma_gather calls per supertile
CQ = SLOT // GQ          # gathered slots per partition per call = 60
NIDX = CQ * PT           # indices per gather call = 7680


# ---------------------------------------------------------------------------
# host-side constant tables (ladder recurrence + quadratic-form weights)
# ---------------------------------------------------------------------------
def _dfact(n):
    r = 1
    while n > 1:
        r *= n
        n -= 2
    return r


def _a_norm(l, m):
    if m == 0:
        return 1.0
    return math.sqrt(2.0 * math.factorial(l - m) / math.factorial(l + m))


def _ladder_coeffs():
    """Monic z-ladder: A~_m = 1, A~_{m+1} = z, A~_l = z A~_{l-1} + gt A~_{l-2};
    Ybar_lm = sig_lm * A~_lm * trig_m. Returns gt[(l,m)], sig[(l,m)]."""
    gt, sig = {}, {}
    for m in range(L + 1):
        k = {m: 1.0 / _dfact(2 * m - 1)}
        if m + 1 <= L:
            k[m + 1] = k[m] / (2 * m + 1)
        for l in range(m + 2, L + 1):
            beta = (2 * l - 1) / (l - m)
            gam = -(l + m - 1) / (l - m)
            k[l] = k[l - 1] / beta
            gt[(l, m)] = gam * k[l] / k[l - 2]
        for l in range(m, L + 1):
            sig[(l, m)] = _a_norm(l, m) / k[l]
    return gt, sig


def _cheb_to_legendre():
    from numpy.polynomial import legendre as npleg, chebyshev as npcheb

    lam = np.zeros((NANG, L + 1))
    for t in range(NANG):
        c = np.zeros(t + 1)
        c[t] = 1.0
        lam[t, : t + 1] = npleg.poly2leg(npcheb.cheb2poly(c))[: t + 1]
    return lam


LAM = _cheb_to_legendre()
GT, SIG = _ladder_coeffs()


def _const_tables():
    # ccoef f16 [81]: gt at slot l*9+m (l-major), 0 elsewhere
    ccoef = np.zeros(81, np.float16)
    for (l, m), v in GT.items():
        ccoef[l * 9 + m] = np.float16(v)
    # gam f32 [162]: sig^2 at rect slot (l*9+m)*2+t for valid (m<=l), else 0
    gam = np.zeros(NC_RECT, np.float32)
    for l in range(L + 1):
        for m in range(l + 1):
            g = np.float32(SIG[(l, m)]) ** 2
            gam[(l * 9 + m) * 2 + 0] = g
            if m >= 1:
                gam[(l * 9 + m) * 2 + 1] = g
    ident = np.eye(PT, dtype=np.float16)
    return ccoef, gam, ident


def view(ap, off, dims):
    """Free-dim view of a tile AP: keep the partition entry, replace free dims
    with explicit [step, count] pairs, shift the element offset by `off`."""
    base = list(ap.ap[0])
    return bass_rust.AP(ap.tensor, ap.offset + off, [base] + [list(d) for d in dims])


def build_supertile(nc, ctx, s, tl, pself, feat_dram, mix_prev=None):
    """Emit one supertile's compute. tl = dict of persistent tiles.
    mix_prev: emitted on DVE between the ladder and weights phases — fills the
    engine while this supertile waits on sectoral (Pool) and the previous
    supertile's matmuls (PE)."""
    base = s * STA

    pn = tl[f"pn{s % 2}"]
    ps = tl[f"ps{s % 2}"]

    # ---- prep + radial + sectoral, emitted per slot-range (lo, n) so the
    # first supertile can start on the first gather call's half ----
    half_pi = tl["half_pi"]
    Srad = tl["Srad"]
    SEC = tl["SEC"]
    uz = tl["uz"]

    def prep_range(lo, n):
        r012 = tl["r012"]
        r_c = [view(r012[:], c * SLOT + lo, [[1, n]]) for c in range(3)]
        for c in range(3):
            nc.vector.tensor_tensor(
                out=r_c[c],
                in0=view(pn[:], c + 4 * lo, [[4, n]]),
                in1=view(ps[:], c, [[0, n // G], [4, G]]),
                op=Alu.subtract,
            )
        sq012 = tl["sq012"]
        sq = [view(sq012[:], c * SLOT + lo, [[1, n]]) for c in range(3)]
        for c in range(3):
            nc.scalar.activation(sq[c], r_c[c], Act.Square)
        d2 = view(tl["d2"][:], lo, [[1, n]])
        nc.vector.tensor_tensor(out=d2, in0=sq[0], in1=sq[1], op=Alu.add)
        nc.vector.tensor_tensor(out=d2, in0=d2, in1=sq[2], op=Alu.add)
        nc.vector.tensor_scalar_max(d2, d2, 1e-18)
        dd = view(tl["dd"][:], lo, [[1, n]])
        nc.scalar.sqrt(dd, d2)
        rinv = view(tl["rinv"][:], lo, [[1, n]])
        nc.vector.reciprocal(rinv, dd)

        # unit vector: x,y straight into SEC block m=1; z separate (f16)
        nc.vector.tensor_tensor(
            out=view(SEC[:], lo, [[1, n]]), in0=r_c[0], in1=rinv, op=Alu.mult
        )
        nc.vector.tensor_tensor(
            out=view(SEC[:], SLOT + lo, [[1, n]]), in0=r_c[1], in1=rinv, op=Alu.mult
        )
        nc.vector.tensor_tensor(
            out=view(uz[:], lo, [[1, n]]), in0=r_c[2], in1=rinv, op=Alu.mult
        )

        m2 = view(tl["m2"][:], lo, [[1, n]])
        nc.vector.tensor_scalar(
            out=m2, in0=dd, scalar1=MIN_CUT, scalar2=None, op0=Alu.is_gt
        )
        # radial weight wr = fc * mask  (wr = mh*(grad+1), mh = 0.5*mask)
        dcr = view(tl["dcr"][:], lo, [[1, n]])
        nc.vector.tensor_scalar_min(dcr, dd, RAD_CUT)
        grad = view(tl["grad"][:], lo, [[1, n]])
        nc.scalar.activation(
            grad, dcr, Act.Sin, bias=half_pi[:], scale=-math.pi / RAD_CUT
        )
        m1h = view(tl["m1h"][:], lo, [[1, n]])
        nc.vector.tensor_scalar(
            out=m1h, in0=dd, scalar1=RAD_CUT, scalar2=0.5, op0=Alu.is_le, op1=Alu.mult
        )
        nc.vector.tensor_tensor(out=m1h, in0=m1h, in1=m2, op=Alu.mult)
        # S0 = wr (f16), written directly into the radial chain tile
        nc.vector.scalar_tensor_tensor(
            out=view(Srad[:], lo, [[1, n]]),
            in0=grad,
            scalar=1.0,
            in1=m1h,
            op0=Alu.add,
            op1=Alu.mult,
        )
        # angular weight w = fca * mask
        dca = view(tl["dcr"][:], lo, [[1, n]])
        nc.vector.tensor_scalar_min(dca, dd, ANG_CUT)
        gang = view(tl["gang"][:], lo, [[1, n]])
        nc.scalar.activation(
            gang, dca, Act.Sin, bias=half_pi[:], scale=-math.pi / ANG_CUT
        )
        a1h = view(tl["a1h"][:], lo, [[1, n]])
        nc.vector.tensor_scalar(
            out=a1h, in0=dd, scalar1=ANG_CUT, scalar2=0.5, op0=Alu.is_le, op1=Alu.mult
        )
        nc.vector.tensor_tensor(out=a1h, in0=a1h, in1=m2, op=Alu.mult)
        wh = view(tl["wh"][:], lo, [[1, n]])
        nc.vector.scalar_tensor_tensor(
            out=wh, in0=gang, scalar=1.0, in1=a1h, op0=Alu.add, op1=Alu.mult
        )
        snh = view(tl["snh"][:], lo, [[1, n]])
        nc.scalar.copy(snh, view(pn[:], 3 + 4 * lo, [[4, n]]))
        nc.vector.tensor_tensor(
            out=view(tl["wsh"][:], lo, [[1, n]]), in0=wh, in1=snh, op=Alu.mult
        )
        # radial chebyshev argument (f16)
        nc.scalar.activation(
            view(tl["xxh"][:], lo, [[1, n]]), dd, Act.Copy, bias=BX_, scale=AX_
        )
        nc.scalar.activation(
            view(tl["x2h"][:], lo, [[1, n]]), dd, Act.Copy, bias=2 * BX_, scale=2 * AX_
        )

    def radial_range(lo, n):
        xxh, x2h, wh, snh = tl["xxh"], tl["x2h"], tl["wh"], tl["snh"]
        nc.vector.tensor_tensor(
            out=view(Srad[:], SLOT + lo, [[1, n]]),
            in0=view(xxh[:], lo, [[1, n]]),
            in1=view(Srad[:], lo, [[1, n]]),
            op=Alu.mult,
        )
        rtmp = view(tl["rtmp"][:], lo, [[1, n]])
        for t in range(2, NRAD):
            nc.vector.tensor_tensor(
                out=rtmp,
                in0=view(x2h[:], lo, [[1, n]]),
                in1=view(Srad[:], (t - 1) * SLOT + lo, [[1, n]]),
                op=Alu.mult,
            )
            nc.vector.tensor_tensor(
                out=view(Srad[:], t * SLOT + lo, [[1, n]]),
                in0=rtmp,
                in1=view(Srad[:], (t - 2) * SLOT + lo, [[1, n]]),
                op=Alu.subtract,
            )
        # F2 = w^2 appended as radial comp 17 (chain A)
        nc.vector.tensor_tensor(
            out=view(Srad[:], NRAD * SLOT + lo, [[1, n]]),
            in0=view(wh[:], lo, [[1, n]]),
            in1=view(wh[:], lo, [[1, n]]),
            op=Alu.mult,
        )
        # chain B: spin-weighted radial values, comps 18..34
        nc.vector.tensor_tensor(
            out=view(Srad[:], NRADC * SLOT + lo, [[SLOT, NRAD], [1, n]]),
            in0=view(Srad[:], lo, [[SLOT, NRAD], [1, n]]),
            in1=view(snh[:], lo, [[0, NRAD], [1, n]]),
            op=Alu.mult,
        )

    def sectoral_range(lo, n):
        tc_ = view(tl["tc_"][:], lo, [[1, n]])
        td_ = view(tl["td_"][:], lo, [[1, n]])
        ux_v = view(SEC[:], lo, [[1, n]])
        uy_v = view(SEC[:], SLOT + lo, [[1, n]])
        for m in range(2, L + 1):
            cp = (m - 2) * 2 * SLOT + lo
            sp = cp + SLOT
            cm = (m - 1) * 2 * SLOT + lo
            sm = cm + SLOT
            nc.gpsimd.tensor_tensor(out=tc_, in0=ux_v, in1=view(SEC[:], cp, [[1, n]]), op=Alu.mult)
            nc.gpsimd.tensor_tensor(out=td_, in0=uy_v, in1=view(SEC[:], sp, [[1, n]]), op=Alu.mult)
            nc.gpsimd.tensor_tensor(
                out=view(SEC[:], cm, [[1, n]]), in0=tc_, in1=td_, op=Alu.subtract
            )
            nc.gpsimd.tensor_tensor(out=tc_, in0=ux_v, in1=view(SEC[:], sp, [[1, n]]), op=Alu.mult)
            nc.gpsimd.tensor_tensor(out=td_, in0=uy_v, in1=view(SEC[:], cp, [[1, n]]), op=Alu.mult)
            nc.gpsimd.tensor_tensor(
                out=view(SEC[:], sm, [[1, n]]), in0=tc_, in1=td_, op=Alu.add
            )

    if s == 0:
        # supertile 0: prep per gather-half so compute starts on the first call
        prep_range(0, CQ)
        prep_range(CQ, CQ)
        radial_range(0, SLOT)
        sectoral_range(0, SLOT)
    else:
        prep_range(0, SLOT)
        radial_range(0, SLOT)
        sectoral_range(0, SLOT)

    # ---- z-ladder (f16, l-major LAD: slot (l*9+m)*SLOT) ----
    LAD = tl["LAD"]
    ccoef = tl["ccoef"]
    # l = m+1 diagonal row: A~_{m+1,m} = z for m=0..7 (slots m*10+9)
    nc.vector.tensor_copy(
        out=view(LAD[:], 9 * SLOT, [[10 * SLOT, 8], [1, SLOT]]),
        in_=view(uz[:], 0, [[0, 8], [1, SLOT]]),
    )
    lt = tl["lt"]
    for l in range(2, L + 1):
        nm = l - 1  # m = 0..l-2
        nc.vector.tensor_tensor(
            out=view(LAD[:], l * 9 * SLOT, [[SLOT, nm], [1, SLOT]]),
            in0=view(uz[:], 0, [[0, nm], [1, SLOT]]),
            in1=view(LAD[:], (l - 1) * 9 * SLOT, [[SLOT, nm], [1, SLOT]]),
            op=Alu.mult,
        )
        nc.vector.tensor_tensor(
            out=view(lt[:], 0, [[SLOT, nm], [1, SLOT]]),
            in0=view(ccoef[:], l * 9, [[1, nm], [0, SLOT]]),
            in1=view(LAD[:], (l - 2) * 9 * SLOT, [[SLOT, nm], [1, SLOT]]),
            op=Alu.mult,
        )
        nc.vector.tensor_tensor(
            out=view(LAD[:], l * 9 * SLOT, [[SLOT, nm], [1, SLOT]]),
            in0=view(LAD[:], l * 9 * SLOT, [[SLOT, nm], [1, SLOT]]),
            in1=view(lt[:], 0, [[SLOT, nm], [1, SLOT]]),
            op=Alu.add,
        )

    # ---- weight tiles WA/WB (f16): (m, trig) slots ----
    WA, WB = tl["WA"], tl["WB"]
    nc.vector.tensor_copy(out=view(WA[:], 0, [[1, SLOT]]), in_=tl["wh"][:])
    nc.vector.tensor_copy(out=view(WB[:], 0, [[1, SLOT]]), in_=tl["wsh"][:])
    for m in range(1, L + 1):
        sec_b = view(SEC[:], (m - 1) * 2 * SLOT, [[SLOT, 2], [1, SLOT]])
        nc.vector.tensor_tensor(
            out=view(WA[:], m * 2 * SLOT, [[SLOT, 2], [1, SLOT]]),
            in0=view(tl["wh"][:], 0, [[0, 2], [1, SLOT]]),
            in1=sec_b,
            op=Alu.mult,
        )
        nc.vector.tensor_tensor(
            out=view(WB[:], m * 2 * SLOT, [[SLOT, 2], [1, SLOT]]),
            in0=view(tl["wsh"][:], 0, [[0, 2], [1, SLOT]]),
            in1=sec_b,
            op=Alu.mult,
        )

    # ---- products into MP rect (f16): MP[(l*9+m)*2+t] = W[m,t] * A~[l,m] ----
    # m=0 has no sin comp: single-trig product; its sin slots are zeroed once.
    for chain, W in enumerate((WA, WB)):
        MP = tl[f"MP{(2 * s + chain) % 3}"]
        nc.vector.tensor_tensor(
            out=view(MP[:], 0, [[18 * SLOT, 9], [1, SLOT]]),
            in0=view(W[:], 0, [[0, 9], [1, SLOT]]),
            in1=view(LAD[:], 0, [[9 * SLOT, 9], [1, SLOT]]),
            op=Alu.mult,
        )
        for m in range(1, L + 1):
            nl = 9 - m
            nc.vector.tensor_tensor(
                out=view(MP[:], m * 20 * SLOT, [[18 * SLOT, nl], [SLOT, 2], [1, SLOT]]),
                in0=view(W[:], m * 2 * SLOT, [[0, nl], [SLOT, 2], [1, SLOT]]),
                in1=view(LAD[:], m * 10 * SLOT, [[9 * SLOT, nl], [0, 2], [1, SLOT]]),
                op=Alu.mult,
            )

    if mix_prev is not None:
        mix_prev()

    # ---- K-reduction on PE: identity-stationary accumulating matmuls ----
    # out per matmul must stay inside one PSUM bank (512 f32): split the 162
    # rect comps into two 81-comp groups per chain at bank-aligned offsets.
    # Radial first so the next supertile's radial chain unblocks earliest.
    ident = tl["ident"]
    accA, accB, accR = tl["accA"], tl["accB"], tl["accR"]
    HC = NC_RECT // 2  # 81
    featt = tl["featt"]
    foff = (s % 2) * G * FEAT
    SQ = tl["SQ"]
    gam = tl["gam"]
    Q = tl["Q"]

    for k in range(K):
        nc.tensor.matmul(
            view(accR[:], 0, [[1, (NRADC + NRAD) * G]]),
            ident[:],
            view(Srad[:], k * G, [[SLOT, NRADC + NRAD], [1, G]]),
            start=(k == 0),
            stop=(k == K - 1),
        )
    # rad_un (f 0..16) and rad_w (f 17..33): iter (t, g) -> featt[g*52 + f]
    nc.scalar.copy(
        out=view(featt[:], foff + 0, [[1, NRAD], [FEAT, G]]),
        in_=view(accR[:], 0, [[G, NRAD], [1, G]]),
    )
    nc.scalar.copy(
        out=view(featt[:], foff + NRAD, [[1, NRAD], [FEAT, G]]),
        in_=view(accR[:], NRADC * G, [[G, NRAD], [1, G]]),
    )
    # F2 (radial comp 17 of chain A) -> F2S[s]
    nc.scalar.copy(
        out=view(tl["F2S"][:], (s % 2) * G, [[1, G]]),
        in_=view(accR[:], NRAD * G, [[1, G]]),
    )

    for ci, acc in enumerate((accA, accB)):
        MP = tl[f"MP{(2 * s + ci) % 3}"]
        for half in range(2):
            for k in range(K):
                nc.tensor.matmul(
                    view(acc[:], half * 512, [[1, HC * G]]),
                    ident[:],
                    view(MP[:], half * HC * SLOT + k * G, [[SLOT, HC], [1, G]]),
                    start=(k == 0),
                    stop=(k == K - 1),
                )


def build_mix(nc, tl, s, feat_dram):
    """B^2 evac + gamma-weight + Q-reduce + lambda-mix + store for supertile s
    (emitted later, while a following supertile's matmuls occupy the PE, so
    the ACT/DVE queues don't stall the next supertile's prep)."""
    SQ, gam, Q, featt = tl["SQ"], tl["gam"], tl["Q"], tl["featt"]
    HC = NC_RECT // 2
    foff = (s % 2) * G * FEAT
    for ci, acc in enumerate((tl["accA"], tl["accB"])):
        soff = ci * NC_RECT * G
        for half in range(2):
            nc.scalar.activation(
                view(SQ[:], soff + half * HC * G, [[1, HC * G]]),
                view(acc[:], half * 512, [[1, HC * G]]),
                Act.Square,
            )
        nc.vector.tensor_tensor(
            out=view(SQ[:], soff, [[1, NC_RECT * G]]),
            in0=view(SQ[:], soff, [[1, NC_RECT * G]]),
            in1=view(gam[:], 0, [[1, NC_RECT], [0, G]]),
            op=Alu.mult,
        )
        nc.vector.tensor_reduce(
            out=view(Q[:], ((s % 2) * 2 + ci) * 9 * G, [[G, 9], [1, G]]),
            in_=view(SQ[:], soff, [[18 * G, 9], [1, G], [G, 18]]),
            axis=AX.X,
            op=Alu.add,
        )
    # ang[t] = sum_l 0.5 lam[t,l] Q_l - 0.5 F2
    F2h = tl["F2h"]
    nc.vector.tensor_scalar(
        out=view(F2h[:], (s % 2) * G, [[1, G]]),
        in0=view(tl["F2S"][:], (s % 2) * G, [[1, G]]),
        scalar1=0.5,
        scalar2=None,
        op0=Alu.mult,
    )
    mixa, mixb = tl["mixa"], tl["mixb"]
    for chain in range(2):
        fbase = 2 * NRAD + chain * NANG
        for t in range(NANG):
            ls = list(range(t % 2, t + 1, 2))
            acc = None
            for i, l in enumerate(ls):
                qv = view(Q[:], ((s % 2) * 2 + chain) * 9 * G + l * G, [[1, G]])
                lam = 0.5 * float(LAM[t, l])
                last = i == len(ls) - 1
                dst = (
                    view(featt[:], foff + fbase + t, [[FEAT, G]])
                    if last
                    else view((mixb if acc is mixa else mixa)[:], 0, [[1, G]])
                )
                if i == 0:
                    src = view(F2h[:], (s % 2) * G, [[1, G]])
                    op1 = Alu.subtract
                else:
                    src = view((mixa if acc is mixa else mixb)[:], 0, [[1, G]])
                    op1 = Alu.add
                nc.vector.scalar_tensor_tensor(
                    out=dst, in0=qv, scalar=lam, in1=src, op0=Alu.mult, op1=op1
                )
                acc = mixa if (acc is not mixa) else mixb

    nc.sync.dma_start(
        out=feat_dram[s * STA : (s + 1) * STA, :].rearrange("(p g) f -> p (g f)", p=PT),
        in_=view(featt[:], foff, [[1, G * FEAT]]),
    )


def build_program():
    I16 = mybir.dt.int16
    nc = bacc.Bacc("TRN2", target_bir_lowering=False, debug=False)
    pos4 = nc.dram_tensor("pos4", [NPAD, ROWE], F32, kind="ExternalInput").ap()
    idx = nc.dram_tensor(
        "idx", [SUP * GQ * PT, NIDX // 16], I16, kind="ExternalInput"
    ).ap()
    pself = nc.dram_tensor("pself", [NPC, 4], F32, kind="ExternalInput").ap()
    ident_d = nc.dram_tensor("ident", [PT, PT], F16, kind="ExternalInput").ap()
    ccoef_d = nc.dram_tensor("ccoef", [PT, 81], F16, kind="ExternalInput").ap()
    gam_d = nc.dram_tensor("gam", [PT, NC_RECT], F32, kind="ExternalInput").ap()
    feat = nc.dram_tensor("feat", [NPC, FEAT], F32, kind="ExternalOutput").ap()

    with tile.TileContext(nc) as tc, ExitStack() as ctx:
        const = ctx.enter_context(tc.tile_pool(name="const", bufs=1))
        io = ctx.enter_context(tc.tile_pool(name="io", bufs=1))
        kp = ctx.enter_context(tc.tile_pool(name="kspace", bufs=1))
        psum = ctx.enter_context(tc.tile_pool(name="psum", bufs=1, space="PSUM"))

        tl = {}

        def T(pool, name, shape, dtype):
            tl[name] = pool.tile(shape, dtype, name=name, tag=name)
            return tl[name]

        # constants
        T(const, "ident", [PT, PT], F16)
        T(const, "ccoef", [PT, 81], F16)
        T(const, "gam", [PT, NC_RECT], F32)
        T(const, "half_pi", [PT, 1], F32)
        nc.sync.dma_start(out=tl["ident"][:], in_=ident_d)
        nc.sync.dma_start(out=tl["ccoef"][:], in_=ccoef_d)
        nc.sync.dma_start(out=tl["gam"][:], in_=gam_d)
        nc.gpsimd.memset(tl["half_pi"][:], HALF_PI)

        # io (double-buffered via explicit 0/1 tiles; single pnw stage)
        I16 = mybir.dt.int16
        for b in range(2):
            T(io, f"pn{b}", [PT, SLOT * 4], F32)
            T(io, f"ps{b}", [PT, G * 4], F32)
        for b in range(2):
            T(io, f"pnw{b}", [PT, CQ * ROWE], F32)
            T(io, f"idxt{b}", [PT, NIDX // 16], I16)


        # prep f32
        for nm in ("d2", "dd", "rinv", "m2", "dcr", "grad", "m1h", "gang", "a1h"):
            T(kp, nm, [PT, SLOT], F32)
        T(kp, "r012", [PT, 3 * SLOT], F32)
        T(kp, "sq012", [PT, 3 * SLOT], F32)
        # f16 working set
        for nm in ("uz", "wh", "snh", "wsh", "xxh", "x2h", "rtmp", "tc_", "td_"):
            T(kp, nm, [PT, SLOT], F16)
        T(kp, "SEC", [PT, 8 * 2 * SLOT], F16)
        T(kp, "LAD", [PT, 81 * SLOT], F16)
        T(kp, "lt", [PT, 7 * SLOT], F16)
        T(kp, "WA", [PT, NC_RECT // 9 * SLOT], F16)
        T(kp, "WB", [PT, NC_RECT // 9 * SLOT], F16)
        for b in range(3):
            T(kp, f"MP{b}", [PT, NC_RECT * SLOT], F16)
        T(kp, "Srad", [PT, (NRADC + NRAD) * SLOT], F16)
        T(kp, "SQ", [PT, 2 * NC_RECT * G], F16)
        T(kp, "featt", [PT, 2 * G * FEAT], F32)
        T(kp, "F2S", [PT, 2 * G], F32)
        T(kp, "F2h", [PT, 2 * G], F32)
        T(kp, "Q", [PT, 2 * 2 * 9 * G], F32)
        T(kp, "mixa", [PT, SUP * G], F32)
        T(kp, "mixb", [PT, SUP * G], F32)

        # psum accumulators (bank-padded: each matmul target inside one bank)
        T(psum, "accA", [PT, 1024], F32)
        T(psum, "accB", [PT, 1024], F32)
        T(psum, "accR", [PT, 512], F32)

        def memset_mp(MPn):
            # zero invalid MP slots (m > l) and the nonexistent m=0 sin comps
            MP = tl[MPn]
            for m in range(1, L + 1):
                nc.gpsimd.memset(
                    view(MP[:], m * 2 * SLOT, [[18 * SLOT, m], [1, 2 * SLOT]]), 0.0
                )
            nc.gpsimd.memset(view(MP[:], SLOT, [[18 * SLOT, 9], [1, SLOT]]), 0.0)

        def emit_onetime_memsets():
            memset_mp("MP0")
            memset_mp("MP1")
            nc.gpsimd.memset(
                view(tl["LAD"][:], 0, [[10 * SLOT, 9], [1, SLOT]]), 1.0
            )

        def gather(s):
            b = s % 2
            pn = tl[f"pn{b}"]
            for q in range(GQ):
                qb = (s * GQ + q) % 2
                idxt = tl[f"idxt{qb}"]
                pnw = tl[f"pnw{qb}"]
                row0 = (s * GQ + q) * PT
                nc.sync.dma_start(out=idxt[:], in_=idx[row0 : row0 + PT, :])
                nc.gpsimd.dma_gather(
                    out_ap=view(pnw[:], 0, [[ROWE, CQ], [1, ROWE]]),
                    in_ap=pos4,
                    idxs_ap=idxt[:],
                    num_idxs=NIDX,
                    num_idxs_reg=NIDX,
                    elem_size=ROWE,
                    single_packet=False,
                )
                nc.scalar.copy(
                    out=view(pn[:], q * CQ * 4, [[1, CQ * 4]]),
                    in_=view(pnw[:], 0, [[ROWE, CQ], [1, 4]]),
                )
            nc.sync.dma_start(
                out=tl[f"ps{b}"][:],
                in_=pself[s * STA : (s + 1) * STA, :].rearrange(
                    "(p g) c -> p (g c)", p=PT
                ),
            )

        gather(0)
        emit_onetime_memsets()
        for s in range(SUP):
            if s == 1:
                memset_mp("MP2")
            if s + 1 < SUP:
                gather(s + 1)
            mix_prev = (
                (lambda sp=s - 1: build_mix(nc, tl, sp, feat)) if s > 0 else None
            )
            build_supertile(nc, ctx, s, tl, pself, feat, mix_prev=mix_prev)
        build_mix(nc, tl, SUP - 1, feat)

    nc.compile()
    return nc


_NC_CACHE = None


def get_program():
    global _NC_CACHE
    if _NC_CACHE is None:
        _NC_CACHE = build_program()
    return _NC_CACHE


def make_in_maps(positions, species_idx, neighbor_idx):
    pos4 = np.zeros((NPAD, ROWE), np.float32)
    pos4[:N, :3] = positions
    pos4[:N, 3] = 2.0 * species_idx.astype(np.float32) - 1.0
    nbrK = np.zeros((NPAD, K), np.int32)
    nbrK[:N] = neighbor_idx.reshape(N, K)

    ccoef, gam, ident = _const_tables()
    ccoef_t = np.broadcast_to(ccoef, (PT, 81)).copy()
    gam_t = np.broadcast_to(gam, (PT, NC_RECT)).copy()

    c_idx = np.arange(SLOT)
    k_of, g_of = c_idx // G, c_idx % G
    p = np.arange(PT)
    in_maps = []
    for c in range(NCORES):
        cb = c * NPC
        blocks = []
        for s in range(SUP):
            # vals[slot, p] = nbrK[cb + s*STA + p*G + g(slot), k(slot)]
            atoms = cb + s * STA + p[None, :] * G + g_of[:, None]  # [SLOT, PT]
            vals = nbrK[atoms, k_of[:, None]].astype(np.int16)
            for q in range(GQ):
                flat = vals[q * CQ : (q + 1) * CQ, :].reshape(-1)  # i = cc*128+p
                wrapped = flat.reshape(-1, 16).T  # [16, NIDX/16]
                blocks.append(np.tile(wrapped, (PT // 16, 1)))
        idx16 = np.concatenate(blocks, axis=0)  # [SUP*GQ*PT, NIDX/16]
        in_maps.append(
            {
                "pos4": pos4,
                "idx": np.ascontiguousarray(idx16),
                "pself": np.ascontiguousarray(pos4[cb : cb + NPC, :4]),
                "ident": ident,
                "ccoef": ccoef_t,
                "gam": gam_t,
            }
        )
    return in_maps


def run(positions, species_idx, neighbor_idx, trace=False, trace_cores=None):
    nc = get_program()
    in_maps = make_in_maps(positions, species_idx, neighbor_idx)
    res = run_bass_kernel_spmd(
        nc,
        in_maps,
        core_ids=list(range(NCORES)),
        trace=trace,
        trace_cores=trace_cores,
    )
    out = np.concatenate([res.results[c]["feat"] for c in range(NCORES)], axis=0)
    return out[:N], res


def kernel(positions, species_idx, neighbor_idx):
    out, _ = run(positions, species_idx, neighbor_idx, trace=False)
    return out
